# revision 5
# baseline (speedup 1.0000x reference)
"""BitNet 3-layer MLP (B=131072, D=256) on 8 TRN2 NeuronCores, data-parallel.

Per-core shard: 16384 rows. All math f32-exact relative to the reference up to
benign summation-order differences:

  per layer:  LayerNorm(row) -> global-absmax int8 fake-quant -> (+-1 W) matmul
              -> scale (-> relu for layers 1,2)

Key implementation tricks:
  - activations between layers are exact integers (relu of +-1-weight matmul of
    int8 values) stored as int16 in SBUF.
  - quantized activations stored as fp16 with a +1536 offset: fp addition
    rounds to integer (round-half-even == jnp.round) for free; the offset term
    is cancelled by an extra K=1 correction matmul (512 * -3*colsum(wb)).
  - LayerNorm scale factors fold into one tensor_scalar: u16 = r*s1 + t where
    s1 = rstd*127/gamma, t = 1536 - mu*s1.
  - gamma = max|xn| is computed as max(rowmax-mu, mu-rowmin)*rstd from max/min
    trees; the global max is one 32-byte AllGather across the 8 cores.
  - layer scaling beta*gamma/127 cancels in the next LayerNorm, so it is only
    applied in the final layer.
"""
import os
import numpy as np
from contextlib import ExitStack

from concourse import bass, tile, mybir
from concourse import bacc
from concourse.bass_utils import run_bass_kernel_spmd
from concourse import bass_isa

P = 128
D = 256
NCORES = 8
B = 131072
B_LOC = B // NCORES          # 16384
T = B_LOC // P               # 128 tiles
G = 8                        # tiles per group
NGRP = T // G                # 16 groups
OFF = 1536.0                 # fp16 rounding offset
LN_EPS = 1e-5
QB = 127.0

f32 = mybir.dt.float32
f16 = mybir.dt.float16
i16 = mybir.dt.int16
Alu = mybir.AluOpType
Act = mybir.ActivationFunctionType


def build_nc():
    nc = bacc.Bacc("TRN2", target_bir_lowering=False, debug=False,
                   num_devices=NCORES)

    x_d = nc.dram_tensor("x", [B_LOC, D], f32, kind="ExternalInput")
    w_d = [nc.dram_tensor(f"W{i+1}", [D, D], f32, kind="ExternalInput")
           for i in range(3)]
    out_d = nc.dram_tensor("out", [B_LOC, D], f32, kind="ExternalOutput")

    with tile.TileContext(nc) as tc:
        with ExitStack() as ctx:
            wt = ctx.enter_context(tc.tile_pool(name="wt", bufs=1))
            stats = ctx.enter_context(tc.tile_pool(name="stats", bufs=2))
            xr1 = ctx.enter_context(tc.tile_pool(name="xr1", bufs=3))
            xr2 = ctx.enter_context(tc.tile_pool(name="xr2", bufs=3))
            u16p = ctx.enter_context(tc.tile_pool(name="u16p", bufs=4))
            u16tp = ctx.enter_context(tc.tile_pool(name="u16tp", bufs=4))
            treep = ctx.enter_context(tc.tile_pool(name="treep", bufs=3))
            smallp = ctx.enter_context(tc.tile_pool(name="smallp", bufs=2))
            psum = ctx.enter_context(tc.tile_pool(name="psum", bufs=3, space="PSUM"))
            psx = ctx.enter_context(tc.tile_pool(name="psx", bufs=1, space="PSUM"))
            dram = ctx.enter_context(tc.tile_pool(name="dram", bufs=2, space="DRAM"))

            arena = wt.tile([P, NGRP, G, D], i16)

            # ---------------- constants ----------------
            ones16 = wt.tile([P, 1], f16)
            nc.vector.memset(ones16[:], 1.0)
            onesf = wt.tile([P, 1], f32)
            nc.vector.memset(onesf[:], 1.0)
            repl = wt.tile([1, P], f32)          # K=1 replicate row
            nc.vector.memset(repl[:], 1.0)
            c512 = wt.tile([1, P], f16)
            nc.vector.memset(c512[:], 512.0)
            epst = wt.tile([P, 1], f32)
            nc.vector.memset(epst[:], LN_EPS)

            # ---------------- weight prep ----------------
            wqT = []     # [128, 2, 256] fp16: wqT[d_in_band, band, j]
            corr = []    # [1, 256] fp16: -3*colsum_wb
            beta = []    # [1, 1] f32
            for li in range(3):
                wf = wt.tile([P, 2, D], f32, tag="wf")
                nc.sync.dma_start(out=wf[:], in_=w_d[li][:].rearrange(
                    "(a p) d -> p a d", p=P))
                # alpha = mean(W)
                rs = wt.tile([P, 2], f32, tag="rs")
                nc.vector.tensor_reduce(out=rs[:], in_=wf[:],
                                        axis=mybir.AxisListType.X, op=Alu.add)
                rv = wt.tile([P, 1], f32, tag="rv")
                nc.vector.tensor_tensor(out=rv[:], in0=rs[:, 0:1], in1=rs[:, 1:2],
                                        op=Alu.add)
                aps = psx.tile([2, P], f32, tag="psx")
                nc.tensor.matmul(aps[0:1, 0:1], lhsT=onesf[:], rhs=rv[:],
                                 start=True, stop=True)
                alpha = wt.tile([1, 1], f32, tag="alpha")
                nc.scalar.activation(out=alpha[:], in_=aps[0:1, 0:1], func=Act.Copy,
                                     scale=1.0 / (D * D))
                abc_ps = psx.tile([P, 1], f32, tag="psx")
                nc.tensor.matmul(abc_ps[:], lhsT=repl[:], rhs=alpha[:],
                                 start=True, stop=True)
                abc = wt.tile([P, 1], f32, tag="abc")
                nc.vector.tensor_copy(out=abc[:], in_=abc_ps[:])
                wc = wt.tile([P, 2, D], f32, tag="wc")
                nc.vector.tensor_scalar(out=wc[:], in0=wf[:], scalar1=abc[:],
                                        scalar2=None, op0=Alu.subtract)
                # beta = mean|wc|
                ba = wt.tile([P, 2], f32, tag="ba")
                nc.vector.tensor_reduce(out=ba[:], in_=wc[:],
                                        axis=mybir.AxisListType.X, op=Alu.add,
                                        apply_absolute_value=True)
                bv = wt.tile([P, 1], f32, tag="bv")
                nc.vector.tensor_tensor(out=bv[:], in0=ba[:, 0:1], in1=ba[:, 1:2],
                                        op=Alu.add)
                bps = psx.tile([2, P], f32, tag="psx")
                nc.tensor.matmul(bps[0:1, 0:1], lhsT=onesf[:], rhs=bv[:],
                                 start=True, stop=True)
                bt = wt.tile([1, 1], f32, tag=f"beta{li}")
                nc.scalar.activation(out=bt[:], in_=bps[0:1, 0:1], func=Act.Copy,
                                     scale=1.0 / (D * D))
                beta.append(bt)
                # wq = sign(wc) in fp16
                wq16 = wt.tile([P, 2, D], f16, tag="wq16")
                nc.vector.tensor_scalar(out=wq16[:], in0=wc[:], scalar1=0.0,
                                        scalar2=2.0, op0=Alu.is_gt, op1=Alu.mult)
                nc.vector.tensor_scalar(out=wq16[:], in0=wq16[:], scalar1=1.0,
                                        scalar2=None, op0=Alu.subtract)
                wqt = wt.tile([P, 2, D], f16, tag=f"wqT{li}")
                for a in range(2):
                    for k in range(2):
                        nc.sync.dma_start_transpose(
                            out=wqt[:, k, a * P:(a + 1) * P],
                            in_=wq16[:, a, k * P:(k + 1) * P])
                wqT.append(wqt)
                cps = psx.tile([1, D], f32, tag="psx")
                nc.tensor.matmul(cps[:], lhsT=ones16[:], rhs=wqt[:, 0, :],
                                 start=True, stop=False)
                nc.tensor.matmul(cps[:], lhsT=ones16[:], rhs=wqt[:, 1, :],
                                 start=False, stop=True)
                cr = wt.tile([1, D], f16, tag=f"corr{li}")
                nc.scalar.activation(out=cr[:], in_=cps[:], func=Act.Copy,
                                     scale=-3.0)
                corr.append(cr)

            # ---------------- layers ----------------
            for li in range(3):
                dt_a = f32 if li == 0 else i16
                last = li == 2

                # ---- sweep 1: statistics ----
                bnt = stats.tile([P, T, 6], f32, tag="bnt")
                rmx = stats.tile([P, T], f32, tag="rmx")
                rmn = stats.tile([P, T], f32, tag="rmn")
                xgrp_l1 = []
                for g in range(NGRP):
                    if li == 0:
                        xg = xr1.tile([P, G, D], f32, tag="xr1")
                        nc.sync.dma_start(out=xg[:], in_=x_d[
                            g * G * P:(g + 1) * G * P, :].rearrange(
                            "(t p) d -> p t d", p=P))
                        grp = xg
                    else:
                        grp = arena[:, g, :, :]
                    for i in range(G):
                        nc.vector.bn_stats(out=bnt[:, g * G + i, :],
                                           in_=grp[:, i, :])
                    for op, dst in ((Alu.max, rmx), (Alu.min, rmn)):
                        tr = treep.tile([P, G, P], dt_a,
                                        tag=f"tree{'f' if li == 0 else 'i'}")
                        nc.vector.tensor_tensor(out=tr[:], in0=grp[:, :, 0:P],
                                                in1=grp[:, :, P:D], op=op)
                        w = P
                        while w > 2:
                            nc.vector.tensor_tensor(
                                out=tr[:, :, 0:w // 2], in0=tr[:, :, 0:w // 2],
                                in1=tr[:, :, w // 2:w], op=op)
                            w //= 2
                        nc.vector.tensor_tensor(
                            out=dst[:, g * G:(g + 1) * G].rearrange(
                                "p (t o) -> p t o", o=1),
                            in0=tr[:, :, 0:1], in1=tr[:, :, 1:2], op=op)

                # ---- per-row stat math on [P, T] ----
                mu = stats.tile([P, T], f32, tag="mu")
                nc.vector.tensor_tensor(out=mu[:], in0=bnt[:, :, 1],
                                        in1=bnt[:, :, 4], op=Alu.add)
                nc.vector.tensor_scalar(out=mu[:], in0=mu[:], scalar1=0.5,
                                        scalar2=None, op0=Alu.mult)
                dm = stats.tile([P, T], f32, tag="dm")
                nc.vector.tensor_tensor(out=dm[:], in0=bnt[:, :, 1],
                                        in1=bnt[:, :, 4], op=Alu.subtract)
                nc.vector.tensor_tensor(out=dm[:], in0=dm[:], in1=dm[:],
                                        op=Alu.mult)
                var = stats.tile([P, T], f32, tag="var")
                nc.vector.tensor_tensor(out=var[:], in0=bnt[:, :, 2],
                                        in1=bnt[:, :, 5], op=Alu.add)
                nc.vector.tensor_scalar(out=dm[:], in0=dm[:], scalar1=64.0,
                                        scalar2=None, op0=Alu.mult)
                nc.vector.tensor_tensor(out=var[:], in0=var[:], in1=dm[:],
                                        op=Alu.add)
                nc.vector.tensor_scalar(out=var[:], in0=var[:], scalar1=1.0 / D,
                                        scalar2=None, op0=Alu.mult)
                rstd = stats.tile([P, T], f32, tag="rstd")
                nc.scalar.activation(out=rstd[:], in_=var[:], func=Act.Sqrt,
                                     bias=epst[:], scale=1.0)
                nc.vector.reciprocal(out=rstd[:], in_=rstd[:])
                # centered absmax * rstd
                a1 = stats.tile([P, T], f32, tag="a1")
                nc.vector.tensor_tensor(out=a1[:], in0=rmx[:], in1=mu[:],
                                        op=Alu.subtract)
                a2 = stats.tile([P, T], f32, tag="a2")
                nc.vector.tensor_tensor(out=a2[:], in0=mu[:], in1=rmn[:],
                                        op=Alu.subtract)
                nc.vector.tensor_tensor(out=a1[:], in0=a1[:], in1=a2[:],
                                        op=Alu.max)
                nc.vector.tensor_tensor(out=a1[:], in0=a1[:], in1=rstd[:],
                                        op=Alu.mult)
                gl = stats.tile([P, 1], f32, tag="gl")
                nc.vector.tensor_reduce(out=gl[:], in_=a1[:],
                                        axis=mybir.AxisListType.X, op=Alu.max)
                nc.gpsimd.partition_all_reduce(gl[:], gl[:], channels=P,
                                               reduce_op=bass_isa.ReduceOp.max)
                nc.vector.tensor_scalar(out=gl[0:1, :], in0=gl[0:1, :],
                                        scalar1=1e-8, scalar2=None, op0=Alu.max)

                # ---- AllGather of local gamma candidate ----
                snd_sb = smallp.tile([1, 8], f32, tag="snd_sb")
                nc.vector.tensor_copy(out=snd_sb[:],
                                      in_=gl[0:1, 0:1].broadcast_to((1, 8)))
                snd = dram.tile([1, 8], f32, tag="snd")
                rcv = dram.tile([1, 8 * NCORES], f32, tag="rcv")
                nc.sync.dma_start(out=snd[:], in_=snd_sb[:])
                nc.gpsimd.collective_compute(
                    "AllGather", Alu.bypass, ins=[snd[:].opt()],
                    outs=[rcv[:].opt()],
                    replica_groups=[list(range(NCORES))])
                g64 = smallp.tile([1, 8 * NCORES], f32, tag="g64")
                nc.sync.dma_start(out=g64[:], in_=rcv[:])
                gam = smallp.tile([1, 1], f32, tag="gam")
                nc.vector.tensor_reduce(out=gam[:], in_=g64[:],
                                        axis=mybir.AxisListType.X, op=Alu.max)
                gi = smallp.tile([1, 1], f32, tag="gi")
                nc.vector.reciprocal(out=gi[:], in_=gam[:])
                nc.vector.tensor_scalar(out=gi[:], in0=gi[:], scalar1=QB,
                                        scalar2=None, op0=Alu.mult)
                gbc_ps = psx.tile([P, 1], f32, tag="psx")
                nc.tensor.matmul(gbc_ps[:], lhsT=repl[:], rhs=gi[:],
                                 start=True, stop=True)
                gbc = smallp.tile([P, 1], f32, tag="gbc")
                nc.vector.tensor_copy(out=gbc[:], in_=gbc_ps[:])
                s1 = stats.tile([P, T], f32, tag="s1")
                nc.vector.tensor_scalar(out=s1[:], in0=rstd[:], scalar1=gbc[:],
                                        scalar2=None, op0=Alu.mult)
                tp = stats.tile([P, T], f32, tag="tp")
                nc.vector.tensor_tensor(out=tp[:], in0=mu[:], in1=s1[:],
                                        op=Alu.mult)
                nc.vector.tensor_scalar(out=tp[:], in0=tp[:], scalar1=-1.0,
                                        scalar2=OFF, op0=Alu.mult, op1=Alu.add)

                if last:
                    # c3 = beta * gamma / 127, broadcast to [P, 1]
                    c3 = smallp.tile([1, 1], f32, tag="c3")
                    nc.vector.tensor_tensor(out=c3[:], in0=beta[li][:],
                                            in1=gam[:], op=Alu.mult)
                    nc.vector.tensor_scalar(out=c3[:], in0=c3[:], scalar1=1.0 / QB,
                                            scalar2=None, op0=Alu.mult)
                    c3bc_ps = psx.tile([P, 1], f32, tag="psx")
                    nc.tensor.matmul(c3bc_ps[:], lhsT=repl[:], rhs=c3[:],
                                     start=True, stop=True)
                    c3bc = smallp.tile([P, 1], f32, tag="c3bc")
                    nc.vector.tensor_copy(out=c3bc[:], in_=c3bc_ps[:])

                # ---- sweep 2: quantize -> transpose -> matmul -> epilogue ----
                for g in range(NGRP):
                    if li == 0:
                        xg2 = xr2.tile([P, G, D], f32, tag="xr2")
                        nc.sync.dma_start(out=xg2[:], in_=x_d[
                            g * G * P:(g + 1) * G * P, :].rearrange(
                            "(t p) d -> p t d", p=P))
                        src_g = xg2
                    else:
                        src_g = arena[:, g, :, :]

                    ps = psum.tile([P, G // 2, D], f32, tag="mm_ps")
                    ps2 = psum.tile([P, G // 2, D], f32, tag="mm_ps")
                    pss = (ps, ps2)
                    for h in range(2):          # half-groups of 4 tiles
                        st = u16p.tile([P, 4, D], f16, tag="u16")
                        for i in range(4):
                            t = g * G + h * 4 + i
                            nc.gpsimd.tensor_scalar(
                                out=st[:, i, :], in0=src_g[:, h * 4 + i, :],
                                scalar1=s1[:, t:t + 1], scalar2=tp[:, t:t + 1],
                                op0=Alu.mult, op1=Alu.add)
                        hT = u16tp.tile([P, 8, P], f16, tag="u16T")
                        nc.sync.dma_start_transpose(
                            out=hT[:], in_=st[:].rearrange("p a d -> p (a d)"))
                        for i in range(4):
                            sl = pss[h][:, i, :]
                            nc.tensor.matmul(sl, lhsT=hT[:, 2 * i, :],
                                             rhs=wqT[li][:, 0, :],
                                             start=True, stop=False)
                            nc.tensor.matmul(sl, lhsT=hT[:, 2 * i + 1, :],
                                             rhs=wqT[li][:, 1, :],
                                             start=False, stop=False)
                            nc.tensor.matmul(sl, lhsT=c512[:], rhs=corr[li][:],
                                             start=False, stop=True)
                    if not last:
                        for h in range(2):
                            nc.scalar.activation(
                                out=arena[:, g, h * 4:(h + 1) * 4, :],
                                in_=pss[h][:], func=Act.Relu, scale=1.0)
                    else:
                        og = xr2.tile([P, G, D], f32, tag="stage")
                        for h in range(2):
                            nc.scalar.activation(
                                out=og[:, h * 4:(h + 1) * 4, :], in_=pss[h][:],
                                func=Act.Copy, scale=c3bc[:])
                        nc.sync.dma_start(
                            out=out_d[g * G * P:(g + 1) * G * P, :].rearrange(
                                "(t p) d -> p t d", p=P), in_=og[:])

    nc.compile()
    return nc


_NC_CACHE = None


def _get_nc():
    global _NC_CACHE
    if _NC_CACHE is None:
        _NC_CACHE = build_nc()
    return _NC_CACHE


def run(inputs, trace=False, **kw):
    nc = _get_nc()
    x = inputs["x"]
    in_maps = []
    for c in range(NCORES):
        in_maps.append({
            "x": np.ascontiguousarray(x[c * B_LOC:(c + 1) * B_LOC]),
            "W1": inputs["W1"], "W2": inputs["W2"], "W3": inputs["W3"],
        })
    res = run_bass_kernel_spmd(nc, in_maps, core_ids=list(range(NCORES)),
                               trace=trace, **kw)
    out = np.concatenate([r["out"] for r in res.results], axis=0)
    return out, res


def kernel(**inputs):
    out, _ = run(inputs)
    return out
